# revision 37
# baseline (speedup 1.0000x reference)
"""Trainium2 Bass kernel for nn_Bert4Argument (embedding_lookup).

Reference computation:
    gathered = take_along_axis(seq, head_indexes, axis=1)        # [B,L,D]
    pe = pos_embedding[j - pos + 256]                             # [B,L,D]
    fe = where(j == pos, class_embedding[frame], class_embedding[0])
    out = concat([gathered, pe, fe], -1) @ W.T + b                # [B,L,200]

Algebraic decomposition (W = [W1 | W2 | W3] along the 3D axis):
    out[i,j] = S_i[h[i,j]] + P[j - pos_i + 256] + (C[f_i] if j==pos_i else C[0]) + b
    where S_i = seq_i @ W1.T  (the only large matmul, done on device),
          P = pos_embedding @ W2.T, C = class_embedding @ W3.T (tiny, host-folded
          into a lookup table like constant-folding BN into conv weights).

Table rows 0..511 hold P[r] + C[0] + b; rows 512+f hold P[256] + C[f] + b so a
single per-position row index (host-computed from pos/frame) covers both the
positional term and the j==pos frame override.

Device-side per batch: 6 accumulating fp32 matmuls compute S (seq passed
pre-transposed so the contraction dim lands on partitions), a one-hot matrix
built from head_indexes (PE transpose + is_equal) gathers S rows via matmul,
and an indirect DMA fetches the table rows; one vector add fuses them.

Sharding: data-parallel over batch, 8 batches per core on 8 cores.
"""

import numpy as np

try:
    import ml_dtypes

    _MM_NP_DTYPES = {
        "bfloat16": ml_dtypes.bfloat16,
        "float32": np.float32,
    }
except ImportError:  # float32 fallback
    _MM_NP_DTYPES = {"float32": np.float32}

B, L, D = 64, 256, 768
LAB = 200
NCORES = 8
NB = B // NCORES  # batches per core
KC = D // 128  # 6 contraction chunks
JC = L // 128  # 2 row chunks
NCOL = NB * JC  # 16 (batch, row-chunk) units per core
TBL_ROWS = 512 + LAB + 1  # 713

# matmul operand dtype: "bfloat16" (fast PE stream + half DMA) or "float32"
MM_DTYPE = "bfloat16"
# table-window rows and device-side output dtype (host upcasts output to f32)
WIN_DTYPE = "float32"
OUT_DTYPE = "float32"

_PROGRAM_CACHE = {}


def build_program():
    """Build + compile the (SPMD-uniform) Bass program. Cached per process."""
    if "nc" in _PROGRAM_CACHE:
        return _PROGRAM_CACHE["nc"]

    import concourse.bacc as bacc
    import concourse.tile as tile
    from concourse import mybir

    f32 = mybir.dt.float32
    i32 = mybir.dt.int32
    mmdt = getattr(mybir.dt, MM_DTYPE)

    nc = bacc.Bacc(
        "TRN2", target_bir_lowering=False, debug=False, num_devices=NCORES
    )
    # all tensors partition-major: row (i, p) holds that partition's whole
    # contiguous free line, so each DMA emits one descriptor per partition
    seqt = nc.dram_tensor(
        "seqt", [NB * 128, KC * L], mmdt, kind="ExternalInput"
    ).ap()
    w1t = nc.dram_tensor("w1t", [128, KC * LAB], mmdt, kind="ExternalInput").ap()
    win = nc.dram_tensor(
        "win", [NB * 128, JC * LAB], getattr(mybir.dt, WIN_DTYPE),
        kind="ExternalInput",
    ).ap()
    cst = nc.dram_tensor(
        "cst", [128, NCOL + JC + 128], f32, kind="ExternalInput"
    ).ap()
    out = nc.dram_tensor(
        "out", [NB * 128, JC * LAB], getattr(mybir.dt, OUT_DTYPE),
        kind="ExternalOutput",
    ).ap()

    with tile.TileContext(nc) as tc:
        _emit(nc, tc, mybir, seqt, w1t, win, cst, out)
    nc.compile()

    _PROGRAM_CACHE["nc"] = nc
    return nc


def _emit(nc, tc, mybir, seqt, w1t, win, cst, out):
    import concourse.bass as bass

    f32 = mybir.dt.float32
    i32 = mybir.dt.int32
    mmdt = getattr(mybir.dt, MM_DTYPE)

    with (
        tc.tile_pool(name="const", bufs=1) as cpool,
        tc.tile_pool(name="work", bufs=4) as work,
        tc.tile_pool(name="ps_s", bufs=3, space="PSUM") as ps_s,
        tc.tile_pool(name="ps_ht", bufs=2, space="PSUM") as ps_ht,
        tc.tile_pool(name="ps_g", bufs=3, space="PSUM") as ps_g,
    ):
        # batch-0 seq + weights first so the first matmul starts ASAP
        sts = []
        for i in range(NB):
            st = work.tile([128, KC, L], mmdt, name=f"st{i}", tag="st", bufs=6)
            sts.append(st)
        # cst first: it is tiny and unblocks the hoisted PE/DVE one-hot work
        cst_sb = cpool.tile([128, NCOL + JC + 128], f32)
        nc.sync.dma_start(cst_sb[:], cst[:])
        nc.sync.dma_start(sts[0][:], seqt[0:128, :].rearrange("p (kc j) -> p kc j", kc=KC))
        w1t_sb = cpool.tile([128, KC, LAB], mmdt)
        nc.sync.dma_start(w1t_sb[:], w1t[:].rearrange("p (kc c) -> p kc c", kc=KC))
        hf_sb = cst_sb[:, 0:NCOL]
        lot_sb = cst_sb[:, NCOL : NCOL + JC]
        ident_sb = cst_sb[:, NCOL + JC : NCOL + JC + 128]

        # Gather one-hot matrices depend only on consts: build all of them up
        # front so the PE/DVE have work during the DMA-dominated head (also
        # warms the PE HAM clock before the main matmul stream).
        def build_gt(col):
            htp = ps_ht.tile([128, 128], f32, name=f"ht{col}", tag="ht")
            nc.tensor.transpose(
                htp[:],
                hf_sb[:, col : col + 1].to_broadcast([128, 128]),
                ident_sb[:],
            )
            gt = cpool.tile([128, JC, 128], mmdt, name=f"gt{col}", tag=f"gt{col}")
            for lc in range(JC):
                nc.vector.tensor_tensor(
                    out=gt[:, lc, :],
                    in0=htp[:],
                    in1=lot_sb[:, lc : lc + 1].to_broadcast([128, 128]),
                    op=mybir.AluOpType.is_equal,
                )
            return gt

        NHOIST = 4
        gts = {col: build_gt(col) for col in range(NHOIST)}

        for i in range(NB):
            st = sts[i]
            if i > 0:
                nc.sync.dma_start(
                    st[:],
                    seqt[i * 128 : (i + 1) * 128, :].rearrange(
                        "p (kc j) -> p kc j", kc=KC
                    ),
                )
            # host-built table window rows for this batch, one DMA
            tg = work.tile(
                [128, JC, LAB], getattr(mybir.dt, WIN_DTYPE), name=f"tg{i}", tag="tg"
            )
            nc.sync.dma_start(
                tg[:],
                win[i * 128 : (i + 1) * 128, :].rearrange(
                    "p (jc c) -> p jc c", jc=JC
                ),
            )
            # S_i = seq_i @ W1.T -> [256, 200], kept as 2 chunks [128, 200]
            ssb = work.tile([128, JC, LAB], mmdt, name=f"ssb{i}", tag="ssb")
            for jc in range(JC):
                spsum = ps_s.tile([128, LAB], f32, name=f"sp{i}_{jc}", tag="sp")
                for kc in range(KC):
                    nc.tensor.matmul(
                        spsum[:],
                        lhsT=st[:, kc, 128 * jc : 128 * jc + 128],
                        rhs=w1t_sb[:, kc, :],
                        start=(kc == 0),
                        stop=(kc == KC - 1),
                    )
                nc.vector.tensor_copy(ssb[:, jc, :], spsum[:])

            ob = work.tile(
                [128, JC, LAB], getattr(mybir.dt, OUT_DTYPE), name=f"ob{i}", tag="ob"
            )
            for jc in range(JC):
                col = JC * i + jc
                if col not in gts:
                    gts[col] = build_gt(col)
                # gathered rows: gp[j, c] = S_i[h[j], c]
                gp = ps_g.tile([128, LAB], f32, name=f"gp{i}_{jc}", tag="gp")
                for lc in range(JC):
                    nc.tensor.matmul(
                        gp[:],
                        lhsT=gts[col][:, lc, :],
                        rhs=ssb[:, lc, :],
                        start=(lc == 0),
                        stop=(lc == JC - 1),
                    )
                nc.vector.tensor_add(
                    out=ob[:, jc, :], in0=gp[:], in1=tg[:, jc, :]
                )
            # store both row-chunks of the batch in one DMA on the Act ring
            nc.scalar.dma_start(
                out[i * 128 : (i + 1) * 128, :].rearrange(
                    "p (jc c) -> p jc c", jc=JC
                ),
                ob[:],
            )


def make_tables(pos_embedding, class_embedding, W, b):
    """Host-side constant folding of the small embedding/classifier terms."""
    pe = np.asarray(pos_embedding, dtype=np.float32)
    ce = np.asarray(class_embedding, dtype=np.float32)
    W = np.asarray(W, dtype=np.float32)
    b = np.asarray(b, dtype=np.float32)
    W1, W2, W3 = W[:, :D], W[:, D : 2 * D], W[:, 2 * D :]
    P = pe @ W2.T  # [513, 200]
    C = ce @ W3.T  # [201, 200]
    tbl = np.empty((TBL_ROWS, LAB), np.float32)
    tbl[:512] = P[:512] + C[0] + b
    tbl[512:] = P[256] + C + b
    # W1.T partition-major: [128, KC*LAB]
    w1t = (
        np.ascontiguousarray(W1.T.reshape(KC, 128, LAB).transpose(1, 0, 2))
        .reshape(128, KC * LAB)
        .astype(_MM_NP_DTYPES[MM_DTYPE])
    )
    return tbl, w1t


def make_core_inputs(core, seq, tbl, w1t, h, fr, pos):
    """Per-core input map (core handles batches [core*NB, core*NB+NB))."""
    i0 = core * NB
    # [NB, L, D] -> [NB, KC, 128, L] -> partition-major [NB, 128, KC, L]
    seqT = (
        np.ascontiguousarray(
            seq[i0 : i0 + NB]
            .transpose(0, 2, 1)
            .reshape(NB, KC, 128, L)
            .transpose(0, 2, 1, 3)
        )
        .reshape(NB * 128, KC * L)
        .astype(_MM_NP_DTYPES[MM_DTYPE])
    )
    idxA = np.empty((128, NCOL), np.int32)
    hfA = np.empty((128, NCOL), np.float32)
    p = np.arange(128)
    for i in range(NB):
        pi = int(pos[i0 + i])
        fi = int(fr[i0 + i])
        for jc in range(JC):
            col = JC * i + jc
            j = 128 * jc + p
            idxA[:, col] = np.where(j == pi, 512 + fi, 256 - pi + j)
            hfA[:, col] = h[i0 + i, j].astype(np.float32)
    ident = np.eye(128, dtype=np.float32)
    lot = np.stack(
        [np.arange(128, dtype=np.float32) + 128 * lc for lc in range(JC)], axis=1
    )
    cstA = np.concatenate([hfA, lot, ident], axis=1)
    # pre-slid table windows, partition-major: row (i, p) = [tbl[idx[p, 2i]] | tbl[idx[p, 2i+1]]]
    winA = np.ascontiguousarray(
        tbl[idxA.T.reshape(NB, JC, 128)].transpose(0, 2, 1, 3)
    ).reshape(NB * 128, JC * LAB).astype(
        _MM_NP_DTYPES.get(WIN_DTYPE, np.float32)
    )
    return {
        "seqt": seqT,
        "w1t": w1t,
        "win": winA,
        "cst": cstA,
    }


def make_in_maps(sequence_output, pos_embedding, class_embedding, W, b,
                 head_indexes, frame, pos):
    seq = np.asarray(sequence_output, dtype=np.float32)
    h = np.asarray(head_indexes).astype(np.int64)
    fr = np.asarray(frame).astype(np.int64)
    posA = np.asarray(pos).astype(np.int64)
    tbl, w1t = make_tables(pos_embedding, class_embedding, W, b)
    return [
        make_core_inputs(c, seq, tbl, w1t, h, fr, posA) for c in range(NCORES)
    ]


def assemble_output(results):
    outs = [
        results[c]["out"]
        .astype(np.float32)
        .reshape(NB, 128, JC, LAB)
        .transpose(0, 2, 1, 3)
        .reshape(NB, L, LAB)
        for c in range(NCORES)
    ]
    return np.concatenate(outs, axis=0)


def kernel(sequence_output, pos_embedding, class_embedding, W, b,
           head_indexes, frame, pos):
    from concourse import bass_utils

    in_maps = make_in_maps(
        sequence_output, pos_embedding, class_embedding, W, b,
        head_indexes, frame, pos,
    )
    nc = build_program()
    res = bass_utils.run_bass_kernel_spmd(
        nc, in_maps, core_ids=list(range(NCORES))
    )
    return assemble_output(res.results)


# revision 38
# speedup vs baseline: 1.0750x; 1.0750x over previous
"""Trainium2 Bass kernel for nn_Bert4Argument (embedding_lookup).

Reference computation:
    gathered = take_along_axis(seq, head_indexes, axis=1)        # [B,L,D]
    pe = pos_embedding[j - pos + 256]                             # [B,L,D]
    fe = where(j == pos, class_embedding[frame], class_embedding[0])
    out = concat([gathered, pe, fe], -1) @ W.T + b                # [B,L,200]

Algebraic decomposition (W = [W1 | W2 | W3] along the 3D axis):
    out[i,j] = S_i[h[i,j]] + P[j - pos_i + 256] + (C[f_i] if j==pos_i else C[0]) + b
    where S_i = seq_i @ W1.T  (the only large matmul, done on device),
          P = pos_embedding @ W2.T, C = class_embedding @ W3.T (tiny, host-folded
          into a lookup table like constant-folding BN into conv weights).

Table rows 0..511 hold P[r] + C[0] + b; rows 512+f hold P[256] + C[f] + b so a
single per-position row index (host-computed from pos/frame) covers both the
positional term and the j==pos frame override.

Device-side per batch: 6 accumulating fp32 matmuls compute S (seq passed
pre-transposed so the contraction dim lands on partitions), a one-hot matrix
built from head_indexes (PE transpose + is_equal) gathers S rows via matmul,
and an indirect DMA fetches the table rows; one vector add fuses them.

Sharding: data-parallel over batch, 8 batches per core on 8 cores.
"""

import numpy as np

try:
    import ml_dtypes

    _MM_NP_DTYPES = {
        "bfloat16": ml_dtypes.bfloat16,
        "float32": np.float32,
    }
except ImportError:  # float32 fallback
    _MM_NP_DTYPES = {"float32": np.float32}

B, L, D = 64, 256, 768
LAB = 200
NCORES = 8
NB = B // NCORES  # batches per core
KC = D // 128  # 6 contraction chunks
JC = L // 128  # 2 row chunks
NCOL = NB * JC  # 16 (batch, row-chunk) units per core
TBL_ROWS = 512 + LAB + 1  # 713

# matmul operand dtype: "bfloat16" (fast PE stream + half DMA) or "float32"
MM_DTYPE = "bfloat16"
# table-window rows and device-side output dtype (host upcasts output to f32)
WIN_DTYPE = "float32"
OUT_DTYPE = "float32"

_PROGRAM_CACHE = {}


def build_program():
    """Build + compile the (SPMD-uniform) Bass program. Cached per process."""
    if "nc" in _PROGRAM_CACHE:
        return _PROGRAM_CACHE["nc"]

    import concourse.bacc as bacc
    import concourse.tile as tile
    from concourse import mybir

    f32 = mybir.dt.float32
    i32 = mybir.dt.int32
    mmdt = getattr(mybir.dt, MM_DTYPE)

    nc = bacc.Bacc(
        "TRN2", target_bir_lowering=False, debug=False, num_devices=NCORES
    )
    # all tensors partition-major: row (i, p) holds that partition's whole
    # contiguous free line, so each DMA emits one descriptor per partition
    seqt = nc.dram_tensor(
        "seqt", [NB * 128, KC * L], mmdt, kind="ExternalInput"
    ).ap()
    w1t = nc.dram_tensor("w1t", [128, KC * LAB], mmdt, kind="ExternalInput").ap()
    win = nc.dram_tensor(
        "win", [NB * 128, JC * LAB], getattr(mybir.dt, WIN_DTYPE),
        kind="ExternalInput",
    ).ap()
    cst = nc.dram_tensor(
        "cst", [128, NCOL + JC + 128], f32, kind="ExternalInput"
    ).ap()
    out = nc.dram_tensor(
        "out", [NB * 128, JC * LAB], getattr(mybir.dt, OUT_DTYPE),
        kind="ExternalOutput",
    ).ap()

    with tile.TileContext(nc) as tc:
        _emit(nc, tc, mybir, seqt, w1t, win, cst, out)
    nc.compile()

    _PROGRAM_CACHE["nc"] = nc
    return nc


def _emit(nc, tc, mybir, seqt, w1t, win, cst, out):
    import concourse.bass as bass

    f32 = mybir.dt.float32
    i32 = mybir.dt.int32
    mmdt = getattr(mybir.dt, MM_DTYPE)

    with (
        tc.tile_pool(name="const", bufs=1) as cpool,
        tc.tile_pool(name="work", bufs=4) as work,
        tc.tile_pool(name="ps_s", bufs=3, space="PSUM") as ps_s,
        tc.tile_pool(name="ps_ht", bufs=2, space="PSUM") as ps_ht,
        tc.tile_pool(name="ps_g", bufs=3, space="PSUM") as ps_g,
    ):
        # batch-0 seq + weights first so the first matmul starts ASAP
        sts = []
        for i in range(NB):
            st = work.tile([128, KC, L], mmdt, name=f"st{i}", tag="st", bufs=6)
            sts.append(st)
        # cst first: it is tiny and unblocks the hoisted PE/DVE one-hot work
        cst_sb = cpool.tile([128, NCOL + JC + 128], f32)
        nc.sync.dma_start(cst_sb[:], cst[:])
        nc.sync.dma_start(sts[0][:], seqt[0:128, :].rearrange("p (kc j) -> p kc j", kc=KC))
        w1t_sb = cpool.tile([128, KC, LAB], mmdt)
        nc.sync.dma_start(w1t_sb[:], w1t[:].rearrange("p (kc c) -> p kc c", kc=KC))
        hf_sb = cst_sb[:, 0:NCOL]
        lot_sb = cst_sb[:, NCOL : NCOL + JC]
        ident_sb = cst_sb[:, NCOL + JC : NCOL + JC + 128]

        # Gather one-hot matrices depend only on consts: build all of them up
        # front so the PE/DVE have work during the DMA-dominated head (also
        # warms the PE HAM clock before the main matmul stream).
        def build_gt(col):
            htp = ps_ht.tile([128, 128], f32, name=f"ht{col}", tag="ht")
            nc.tensor.transpose(
                htp[:],
                hf_sb[:, col : col + 1].to_broadcast([128, 128]),
                ident_sb[:],
            )
            gt = cpool.tile([128, JC, 128], mmdt, name=f"gt{col}", tag=f"gt{col}")
            for lc in range(JC):
                nc.vector.tensor_tensor(
                    out=gt[:, lc, :],
                    in0=htp[:],
                    in1=lot_sb[:, lc : lc + 1].to_broadcast([128, 128]),
                    op=mybir.AluOpType.is_equal,
                )
            return gt

        gts = {col: build_gt(col) for col in range(NCOL)}

        for i in range(NB):
            st = sts[i]
            if i > 0:
                nc.sync.dma_start(
                    st[:],
                    seqt[i * 128 : (i + 1) * 128, :].rearrange(
                        "p (kc j) -> p kc j", kc=KC
                    ),
                )
            # host-built table window rows for this batch, one DMA
            tg = work.tile(
                [128, JC, LAB], getattr(mybir.dt, WIN_DTYPE), name=f"tg{i}", tag="tg"
            )
            nc.sync.dma_start(
                tg[:],
                win[i * 128 : (i + 1) * 128, :].rearrange(
                    "p (jc c) -> p jc c", jc=JC
                ),
            )
            # S_i = seq_i @ W1.T -> [256, 200], kept as 2 chunks [128, 200]
            ssb = work.tile([128, JC, LAB], mmdt, name=f"ssb{i}", tag="ssb")
            for jc in range(JC):
                spsum = ps_s.tile([128, LAB], f32, name=f"sp{i}_{jc}", tag="sp")
                for kc in range(KC):
                    nc.tensor.matmul(
                        spsum[:],
                        lhsT=st[:, kc, 128 * jc : 128 * jc + 128],
                        rhs=w1t_sb[:, kc, :],
                        start=(kc == 0),
                        stop=(kc == KC - 1),
                    )
                nc.vector.tensor_copy(ssb[:, jc, :], spsum[:])

            ob = work.tile(
                [128, JC, LAB], getattr(mybir.dt, OUT_DTYPE), name=f"ob{i}", tag="ob"
            )
            for jc in range(JC):
                col = JC * i + jc
                # gathered rows: gp[j, c] = S_i[h[j], c]
                gp = ps_g.tile([128, LAB], f32, name=f"gp{i}_{jc}", tag="gp")
                for lc in range(JC):
                    nc.tensor.matmul(
                        gp[:],
                        lhsT=gts[col][:, lc, :],
                        rhs=ssb[:, lc, :],
                        start=(lc == 0),
                        stop=(lc == JC - 1),
                    )
                nc.vector.tensor_add(
                    out=ob[:, jc, :], in0=gp[:], in1=tg[:, jc, :]
                )
            # store both row-chunks of the batch in one DMA on the Act ring
            nc.scalar.dma_start(
                out[i * 128 : (i + 1) * 128, :].rearrange(
                    "p (jc c) -> p jc c", jc=JC
                ),
                ob[:],
            )


def make_tables(pos_embedding, class_embedding, W, b):
    """Host-side constant folding of the small embedding/classifier terms."""
    pe = np.asarray(pos_embedding, dtype=np.float32)
    ce = np.asarray(class_embedding, dtype=np.float32)
    W = np.asarray(W, dtype=np.float32)
    b = np.asarray(b, dtype=np.float32)
    W1, W2, W3 = W[:, :D], W[:, D : 2 * D], W[:, 2 * D :]
    P = pe @ W2.T  # [513, 200]
    C = ce @ W3.T  # [201, 200]
    tbl = np.empty((TBL_ROWS, LAB), np.float32)
    tbl[:512] = P[:512] + C[0] + b
    tbl[512:] = P[256] + C + b
    # W1.T partition-major: [128, KC*LAB]
    w1t = (
        np.ascontiguousarray(W1.T.reshape(KC, 128, LAB).transpose(1, 0, 2))
        .reshape(128, KC * LAB)
        .astype(_MM_NP_DTYPES[MM_DTYPE])
    )
    return tbl, w1t


def make_core_inputs(core, seq, tbl, w1t, h, fr, pos):
    """Per-core input map (core handles batches [core*NB, core*NB+NB))."""
    i0 = core * NB
    # [NB, L, D] -> [NB, KC, 128, L] -> partition-major [NB, 128, KC, L]
    seqT = (
        np.ascontiguousarray(
            seq[i0 : i0 + NB]
            .transpose(0, 2, 1)
            .reshape(NB, KC, 128, L)
            .transpose(0, 2, 1, 3)
        )
        .reshape(NB * 128, KC * L)
        .astype(_MM_NP_DTYPES[MM_DTYPE])
    )
    idxA = np.empty((128, NCOL), np.int32)
    hfA = np.empty((128, NCOL), np.float32)
    p = np.arange(128)
    for i in range(NB):
        pi = int(pos[i0 + i])
        fi = int(fr[i0 + i])
        for jc in range(JC):
            col = JC * i + jc
            j = 128 * jc + p
            idxA[:, col] = np.where(j == pi, 512 + fi, 256 - pi + j)
            hfA[:, col] = h[i0 + i, j].astype(np.float32)
    ident = np.eye(128, dtype=np.float32)
    lot = np.stack(
        [np.arange(128, dtype=np.float32) + 128 * lc for lc in range(JC)], axis=1
    )
    cstA = np.concatenate([hfA, lot, ident], axis=1)
    # pre-slid table windows, partition-major: row (i, p) = [tbl[idx[p, 2i]] | tbl[idx[p, 2i+1]]]
    winA = np.ascontiguousarray(
        tbl[idxA.T.reshape(NB, JC, 128)].transpose(0, 2, 1, 3)
    ).reshape(NB * 128, JC * LAB).astype(
        _MM_NP_DTYPES.get(WIN_DTYPE, np.float32)
    )
    return {
        "seqt": seqT,
        "w1t": w1t,
        "win": winA,
        "cst": cstA,
    }


def make_in_maps(sequence_output, pos_embedding, class_embedding, W, b,
                 head_indexes, frame, pos):
    seq = np.asarray(sequence_output, dtype=np.float32)
    h = np.asarray(head_indexes).astype(np.int64)
    fr = np.asarray(frame).astype(np.int64)
    posA = np.asarray(pos).astype(np.int64)
    tbl, w1t = make_tables(pos_embedding, class_embedding, W, b)
    return [
        make_core_inputs(c, seq, tbl, w1t, h, fr, posA) for c in range(NCORES)
    ]


def assemble_output(results):
    outs = [
        results[c]["out"]
        .astype(np.float32)
        .reshape(NB, 128, JC, LAB)
        .transpose(0, 2, 1, 3)
        .reshape(NB, L, LAB)
        for c in range(NCORES)
    ]
    return np.concatenate(outs, axis=0)


def kernel(sequence_output, pos_embedding, class_embedding, W, b,
           head_indexes, frame, pos):
    from concourse import bass_utils

    in_maps = make_in_maps(
        sequence_output, pos_embedding, class_embedding, W, b,
        head_indexes, frame, pos,
    )
    nc = build_program()
    res = bass_utils.run_bass_kernel_spmd(
        nc, in_maps, core_ids=list(range(NCORES))
    )
    return assemble_output(res.results)
